# revision 15
# baseline (speedup 1.0000x reference)
# ARFSA attention kernel for 8 TRN2 NeuronCores (Bass/Tile), v7.
#
# Reference computation (per batch b, channel c):
#   q = Wq x + bq ; k = Wk x + bk ; v = Wv x + bv          (1x1 convs)
#   att = softmax_flat( q @ (k + P)^T )                    (P = pos_code, same
#   out = att * v                                           for all channels)
#
# Design (data-parallel over batch, 4 per core):
#   * P and biases folded into the projections via an augmented x
#     (ones-row + P-row), so K+P comes straight out of PSUM.
#   * FUSED q|k|v projection: one 192-column matmul per 128-position
#     chunk (128 matmuls/batch).  The v6 split V projection doubled the
#     PE instruction count with LDW-bound 64-row matmuls; each PE
#     instruction carries ~170-250ns of fixed latency, which made the PE
#     the gating producer for every downstream engine.
#   * x is consumed streaming by the projection (small rotating chunks),
#     freeing SBUF for the fused qkv tile.
#   * Softmax denominators: exp -> bf16 E, VectorE reduce, cross-
#     partition sum via a ones-matmul, reciprocal.
#   * V is rescaled by 1/s_c in an explicit broadcast-multiply pass into
#     a separate contiguous buffer (VectorE, stride-0 broadcast AP).
#   * Final multiply grouped by H: E read through a strided view,
#     Vs read contiguous; groups split GpSimd / VectorE.  pass2 of
#     batch b-2 spreads across the whole iteration; the last batch
#     chases its scale chunks directly (no pipeline tail).
#
# Layouts (per core):
#   xa   DRAM in  [4, 66, 16384] fp16   rows 0..63 = x, row 64 = 1.0 (bias),
#                                       row 65 = P.flatten() (K-only via waug)
#   waug DRAM in  [66, 192] fp16        cols 0:64 Wq^T | 64:128 Wk^T | 128:192 Wv^T
#   out  DRAM out [4, 128(w), 128(h), 64(c)] bf16  (host transposes to [b,c,h,w])

import sys

if "/opt/trn_rl_repo" not in sys.path:
    sys.path.insert(0, "/opt/trn_rl_repo")

import numpy as np
from contextlib import ExitStack

import concourse.bass as bass
import concourse.tile as tile
from concourse import bacc, mybir
from concourse.bass_utils import run_bass_kernel_spmd

N_CORES = 8
B_LOC = 4            # 32 batches / 8 cores
C = 64               # out channels
F = 128              # feature map size
S = F * F            # 16384 positions

FP16 = mybir.dt.float16
BF16 = mybir.dt.bfloat16
F32 = mybir.dt.float32

_BUILT = {}

# Engine split knobs.
EV_ON_ACT = set(range(32)) - {1, 4, 7, 10, 13, 16, 19, 22, 25, 28, 31}  # 21/32
SCALE_ON_GP = {1, 5}                    # of 8 scale chunks (rest VectorE)
PASSB_ON_GPSIMD = {0, 1, 2, 4, 5, 6}    # of 8 pass-B groups (rest VectorE)
ATT_AT = {3, 6, 9, 12, 15, 18, 21, 24}          # g-slots for att groups
PASSB_AT = {1, 5, 9, 13, 17, 21, 25, 29}        # g-slots for pass2 groups


def _build_bass():
    nc = bacc.Bacc("TRN2", target_bir_lowering=False, debug=False)

    xa = nc.declare_dram_parameter("xa", [B_LOC, 66, S], FP16, isOutput=False)
    waug = nc.declare_dram_parameter("waug", [66, 192], FP16, isOutput=False)
    out = nc.declare_dram_parameter("out", [B_LOC, F, F, C], BF16, isOutput=True)

    with ExitStack() as ctx:
        tc = ctx.enter_context(tile.TileContext(nc))

        const = ctx.enter_context(tc.tile_pool(name="const", bufs=1))
        xpool = ctx.enter_context(tc.tile_pool(name="xpool", bufs=3))
        qkvpool = ctx.enter_context(tc.tile_pool(name="qkv", bufs=2))
        epool = ctx.enter_context(tc.tile_pool(name="epool", bufs=2))
        vspool = ctx.enter_context(tc.tile_pool(name="vs", bufs=2))
        rpool = ctx.enter_context(tc.tile_pool(name="rpool", bufs=2))
        opool = ctx.enter_context(tc.tile_pool(name="opool", bufs=4))
        ps = ctx.enter_context(tc.tile_pool(name="ps", bufs=2, space="PSUM"))
        psa = ctx.enter_context(tc.tile_pool(name="psa", bufs=2, space="PSUM"))

        waug_sb = const.tile([66, 192], FP16, tag="waug")
        nc.sync.dma_start(out=waug_sb[:], in_=waug[:, :])
        ones_sb = const.tile([128, 128], BF16, tag="ones")
        nc.gpsimd.memset(ones_sb[:], 1.0)

        st = {}   # per-batch pipeline state

        def emit_proj_group(b, x_t, g, g4):
            s = st.setdefault(b, {})
            if g == 0:
                s["qkv"] = qkvpool.tile([128, F, 192], FP16, tag="qkv",
                                        name=f"qkv_{b}")  # [w, h, q|k|v]
            pt = ps.tile([128, 4, 256], F32, tag="ps", name=f"pt_{b}_{g}")
            for jj in range(4):
                j = g4 * 4 + jj
                nc.tensor.matmul(
                    pt[:, jj, 0:192],
                    lhsT=x_t[:, j * F:(j + 1) * F],
                    rhs=waug_sb[:, :],
                    start=True, stop=True,
                )
            dst = s["qkv"][:, g * 4:(g + 1) * 4, :]
            if g in EV_ON_ACT:
                nc.scalar.copy(dst, pt[:, :, 0:192])
            else:
                nc.vector.tensor_copy(dst, pt[:, :, 0:192])

        def emit_att_group(b, cg):
            s = st[b]
            if cg == 0:
                s["e"] = epool.tile([128, C, F], BF16, tag="e", name=f"e_{b}")
                s["r"] = rpool.tile([128, C], BF16, tag="r", name=f"r_{b}")
            c0 = cg * 8
            at = psa.tile([128, 8, 128], F32, tag="psa", name=f"at_{b}_{cg}")
            for cc in range(8):
                c = c0 + cc
                nc.tensor.matmul(
                    at[:, cc, :],
                    lhsT=s["qkv"][:, :, 64 + c],   # (K+P)^T tile [w, v]
                    rhs=s["qkv"][:, :, c],         # Q^T tile [w, h]
                    start=True, stop=True,
                )
            nc.scalar.activation(
                s["e"][:, c0:c0 + 8, :], at[:, :, :],
                mybir.ActivationFunctionType.Exp,
            )
            with nc.allow_low_precision("bf16 softmax denominators"):
                nc.vector.tensor_reduce(
                    s["r"][:, c0:c0 + 8], s["e"][:, c0:c0 + 8, :],
                    axis=mybir.AxisListType.X, op=mybir.AluOpType.add,
                )

        def emit_sinv(b):
            s = st[b]
            spt = psa.tile([128, 8, 128], F32, tag="psa", name=f"sp_{b}")
            sp = spt[:, 0, 0:64]
            nc.tensor.matmul(sp, lhsT=ones_sb[:], rhs=s["r"][:, :],
                             start=True, stop=True)
            sinv = rpool.tile([128, C], F32, tag="sinv", name=f"sinv_{b}")
            nc.vector.reciprocal(sinv[:, :], sp)
            s["sinv"] = sinv

        def emit_scale_chunk(b, k):
            s = st[b]
            if k == 0:
                s["vs"] = vspool.tile([128, F, C], BF16, tag="vs",
                                      name=f"vs_{b}")  # [w, h, c] scaled V
            h0 = k * 16
            src = s["qkv"][:, h0:h0 + 16, 128:192]
            bc = s["sinv"][:, :].unsqueeze(1).broadcast_to((128, 16, 64))
            eng = nc.gpsimd if k in SCALE_ON_GP else nc.vector
            eng.tensor_mul(s["vs"][:, h0:h0 + 16, :], src, bc)

        def emit_pass2(b, hg):
            s = st[b]
            h0 = hg * 16
            ot = opool.tile([128, 16, C], BF16, tag="ot", name=f"ot_{b}_{hg}")
            # E read through a transposed view: [w, c, h-slice] -> [w, h, c]
            e_view = s["e"][:, :, h0:h0 + 16].transpose([0, 2, 1])
            eng = nc.gpsimd if hg in PASSB_ON_GPSIMD else nc.vector
            eng.tensor_mul(ot[:, :, :], e_view, s["vs"][:, h0:h0 + 16, :])
            nc.sync.dma_start(out=out[b, :, h0:h0 + 16, :], in_=ot[:])

        # ---- software pipeline ----
        # iteration i: proj(i) + att(i-1) + pass2(i-2) interleaved, then
        # sinv(i-1) and the V-scale chunks of batch i-1.  The final batch's
        # pass2 chases its scale chunks directly.
        for i in range(B_LOC + 1):
            p = i if i < B_LOC else None            # projection batch
            c = i - 1 if i >= 1 else None           # att + scale batch
            pb = i - 2 if i >= 2 else None          # spread pass2 batch

            ai = pi = 0
            for g in range(32):
                if p is not None:
                    if g % 4 == 0:
                        xc = g // 4
                        x_t = xpool.tile([66, 2048], FP16, tag="xt",
                                         name=f"xt_{p}_{xc}")
                        nc.sync.dma_start(
                            out=x_t[:],
                            in_=xa[p, :, xc * 2048:(xc + 1) * 2048])
                    emit_proj_group(p, x_t, g, g % 4)
                if c is not None and g in ATT_AT:
                    emit_att_group(c, ai)
                    ai += 1
                if pb is not None and g in PASSB_AT:
                    emit_pass2(pb, pi)
                    pi += 1
            if c is not None:
                emit_sinv(c)
                last = c == B_LOC - 1
                for k in range(8):
                    emit_scale_chunk(c, k)
                    if last:
                        emit_pass2(c, k)

    nc.compile()
    return nc


def _get_built():
    if "nc" not in _BUILT:
        _BUILT["nc"] = _build_bass()
    return _BUILT["nc"]


def _prep_inputs(x, wq, bq, wk, bk, wv, bv, pos_code):
    x = np.asarray(x, np.float32)
    pos = np.asarray(pos_code, np.float32)[0]          # identical across channels
    waug = np.zeros([66, 192], np.float32)
    waug[0:64, 0:64] = np.asarray(wq, np.float32).T
    waug[0:64, 64:128] = np.asarray(wk, np.float32).T
    waug[0:64, 128:192] = np.asarray(wv, np.float32).T
    waug[64, 0:64] = np.asarray(bq, np.float32)
    waug[64, 64:128] = np.asarray(bk, np.float32)
    waug[64, 128:192] = np.asarray(bv, np.float32)
    waug[65, 64:128] = 1.0                             # P-row hits K channels only
    waug16 = waug.astype(np.float16)

    pflat16 = pos.reshape(-1).astype(np.float16)
    xf = x.reshape(x.shape[0], x.shape[1], S)
    in_maps = []
    for core in range(N_CORES):
        xs = xf[core * B_LOC:(core + 1) * B_LOC]
        xa = np.empty([B_LOC, 66, S], np.float16)
        xa[:, 0:64] = xs.astype(np.float16)
        xa[:, 64] = np.float16(1.0)
        xa[:, 65] = pflat16[None, :]
        in_maps.append({"xa": xa, "waug": waug16})
    return in_maps


LAST_RESULTS = None


def kernel(x, wq, bq, wk, bk, wv, bv, pos_code, _trace=False):
    global LAST_RESULTS
    in_maps = _prep_inputs(x, wq, bq, wk, bk, wv, bv, pos_code)
    nc = _get_built()
    res = run_bass_kernel_spmd(nc, in_maps, core_ids=list(range(N_CORES)),
                               trace=_trace)
    LAST_RESULTS = res
    outs = []
    for core in range(N_CORES):
        o = np.asarray(res.results[core]["out"])       # [4, w, h, c] bf16
        outs.append(np.transpose(o.astype(np.float32), (0, 3, 2, 1)))
    return np.concatenate(outs, axis=0)


# revision 16
# speedup vs baseline: 1.1332x; 1.1332x over previous
# ARFSA attention kernel for 8 TRN2 NeuronCores (Bass/Tile), v4.
#
# Reference computation (per batch b, channel c):
#   q = Wq x + bq ; k = Wk x + bk ; v = Wv x + bv          (1x1 convs)
#   att = softmax_flat( q @ (k + P)^T )                    (P = pos_code, same
#   out = att * v                                           for all channels)
#
# Design (data-parallel over batch, 4 per core):
#   * P and biases folded into the projections via an augmented x
#     (ones-row + P-row), so K+P comes straight out of PSUM.
#   * Q,K projected with x-chunks stationary -> tiles come out [w, h, ch];
#     per-channel att matmuls then read [w, 128] slices.
#   * Softmax without max-subtraction (logits bounded ~|45|, fp32 exp,
#     bf16 E storage).
#   * The softmax reciprocal 1/s_c is folded into the V projection
#     WEIGHTS (Wv is [66,64]): V is projected only after the denominator
#     is known, so V comes out of the PE already scaled (wvs must be
#     bf16: sinv can be ~e^-45 which underflows fp16).
#   * All PSUM->SBUF evictions use contiguous destination APs (strided
#     writes measured 5-6 ns/elem) and are split ScalarE/VectorE by knobs.
#   * Final multiply is grouped by H (not C): each group depends on one
#     V-eviction group only, so it pipelines right behind the V
#     projection with no tail.  The E-operand is read through a strided
#     view; groups are split between GpSimd (otherwise idle) and VectorE.
#   * att matmuls interleave into the next batch's QK-projection stream
#     so the in-order PE/Act queues never head-of-line block.
#
# Layouts (per core):
#   xa   DRAM in  [4, 66, 16384] fp16   rows 0..63 = x, row 64 = 1.0 (bias),
#                                       row 65 = P.flatten() (K-only via waug)
#   waug DRAM in  [66, 192] fp16        cols 0:64 Wq^T | 64:128 Wk^T | 128:192 Wv^T
#   out  DRAM out [4, 128(w), 128(h), 64(c)] bf16  (host transposes to [b,c,h,w])

import sys

if "/opt/trn_rl_repo" not in sys.path:
    sys.path.insert(0, "/opt/trn_rl_repo")

import numpy as np
from contextlib import ExitStack

import concourse.bass as bass
import concourse.tile as tile
from concourse import bacc, mybir
from concourse.bass_utils import run_bass_kernel_spmd

N_CORES = 8
B_LOC = 4            # 32 batches / 8 cores
C = 64               # out channels
F = 128              # feature map size
S = F * F            # 16384 positions

FP16 = mybir.dt.float16
BF16 = mybir.dt.bfloat16
F32 = mybir.dt.float32

_BUILT = {}

# Engine split knobs: which group indices go to ScalarE (rest: VectorE).
QK_ON_ACT = set(range(16)) - {2, 5, 8, 11, 14}      # 11 of 16
V_ON_ACT = set(range(16)) - {1, 4, 7, 10, 13, 15}   # 10 of 16
# pass-B h-groups on GpSimd (rest: VectorE, same strided E read).
PASSB_ON_GPSIMD = set(range(16)) - {3, 7, 11, 15}   # 12 of 16


def _build_bass():
    nc = bacc.Bacc("TRN2", target_bir_lowering=False, debug=False)

    xa = nc.declare_dram_parameter("xa", [B_LOC, 66, S], FP16, isOutput=False)
    waug = nc.declare_dram_parameter("waug", [66, 192], FP16, isOutput=False)
    out = nc.declare_dram_parameter("out", [B_LOC, F, F, C], BF16, isOutput=True)

    with ExitStack() as ctx:
        tc = ctx.enter_context(tile.TileContext(nc))

        const = ctx.enter_context(tc.tile_pool(name="const", bufs=1))
        xpool = ctx.enter_context(tc.tile_pool(name="xpool", bufs=2))
        qkpool = ctx.enter_context(tc.tile_pool(name="qkpool", bufs=2))
        epool = ctx.enter_context(tc.tile_pool(name="epool", bufs=2))
        vpool = ctx.enter_context(tc.tile_pool(name="vpool", bufs=2))
        rpool = ctx.enter_context(tc.tile_pool(name="rpool", bufs=2))
        opool = ctx.enter_context(tc.tile_pool(name="opool", bufs=4))
        ps = ctx.enter_context(tc.tile_pool(name="ps", bufs=3, space="PSUM"))
        psv = ctx.enter_context(tc.tile_pool(name="psv", bufs=2, space="PSUM"))

        waug_sb = const.tile([66, 192], FP16, tag="waug")
        nc.sync.dma_start(out=waug_sb[:], in_=waug[:, :])
        ones_sb = const.tile([128, 128], BF16, tag="ones")
        nc.gpsimd.memset(ones_sb[:], 1.0)

        st = {}   # per-batch pipeline state

        def emit_xload(b):
            x_t = xpool.tile([66, S], FP16, tag="xt", name=f"xt_{b}")
            for xc in range(8):
                nc.sync.dma_start(out=x_t[:, xc * 2048:(xc + 1) * 2048],
                                  in_=xa[b, :, xc * 2048:(xc + 1) * 2048])
            st[b] = {"x": x_t}

        def emit_qk_group(b, g):
            s = st[b]
            if g == 0:
                s["qk"] = qkpool.tile([128, F, 128], FP16, tag="qk",
                                      name=f"qk_{b}")  # [w, h, q|k]
            pqk = ps.tile([128, 8, 128], F32, tag="ps", name=f"pqk_{b}_{g}")
            for jj in range(8):
                j = g * 8 + jj
                nc.tensor.matmul(
                    pqk[:, jj, :],
                    lhsT=s["x"][:, j * F:(j + 1) * F],
                    rhs=waug_sb[:, 0:128],
                    start=True, stop=True,
                )
            if g in QK_ON_ACT:
                nc.scalar.copy(s["qk"][:, g * 8:(g + 1) * 8, :], pqk[:, :, :])
            else:
                nc.vector.tensor_copy(s["qk"][:, g * 8:(g + 1) * 8, :],
                                      pqk[:, :, :])

        def emit_att_group(b, cg):
            s = st[b]
            if cg == 0:
                s["e"] = epool.tile([128, C, F], BF16, tag="e", name=f"e_{b}")
                s["r"] = rpool.tile([128, C], BF16, tag="r", name=f"r_{b}")
            c0 = cg * 8
            at = ps.tile([128, 8, 128], F32, tag="ps", name=f"at_{b}_{cg}")
            for cc in range(8):
                c = c0 + cc
                nc.tensor.matmul(
                    at[:, cc, :],
                    lhsT=s["qk"][:, :, 64 + c],   # (K+P)^T tile [w, v]
                    rhs=s["qk"][:, :, c],         # Q^T tile [w, h]
                    start=True, stop=True,
                )
            nc.scalar.activation(
                s["e"][:, c0:c0 + 8, :], at[:, :, :],
                mybir.ActivationFunctionType.Exp,
            )
            with nc.allow_low_precision("bf16 softmax denominators"):
                nc.vector.tensor_reduce(
                    s["r"][:, c0:c0 + 8], s["e"][:, c0:c0 + 8, :],
                    axis=mybir.AxisListType.X, op=mybir.AluOpType.add,
                )

        def emit_sinv(b):
            s = st[b]
            spt = ps.tile([128, 8, 128], F32, tag="ps", name=f"sp_{b}")
            sp = spt[:, 0, 0:64]
            nc.tensor.matmul(sp, lhsT=ones_sb[:], rhs=s["r"][:, :],
                             start=True, stop=True)
            sinv = rpool.tile([128, C], F32, tag="sinv", name=f"sinv_{b}")
            nc.vector.reciprocal(sinv[:, :], sp)
            wvs = rpool.tile([66, C], BF16, tag="wvs", name=f"wvs_{b}")
            nc.vector.tensor_mul(wvs[:, :], waug_sb[0:66, 128:192],
                                 sinv[0:66, :])
            s["wvs"] = wvs

        def emit_v_group(b, vg):
            s = st[b]
            if vg == 0:
                s["v"] = vpool.tile([128, F, C], BF16, tag="v",
                                    name=f"v_{b}")  # [w, h, c] natural layout
            pv = psv.tile([128, 8, 64], F32, tag="psv", name=f"pv_{b}_{vg}")
            for jj in range(8):
                j = vg * 8 + jj
                nc.tensor.matmul(
                    pv[:, jj, :],
                    lhsT=s["x"][:, j * F:(j + 1) * F],
                    rhs=s["wvs"][:, :],
                    start=True, stop=True,
                )
            dst = s["v"][:, vg * 8:(vg + 1) * 8, :]
            if vg in V_ON_ACT:
                nc.scalar.copy(dst, pv[:, :, :])
            else:
                nc.vector.tensor_copy(dst, pv[:, :, :])

        def emit_pass2(b, hg):
            s = st[b]
            h0 = hg * 8
            ot = opool.tile([128, 8, C], BF16, tag="ot", name=f"ot_{b}_{hg}")
            # E read through a transposed view: [w, c, h-slice] -> [w, h, c]
            e_view = s["e"][:, :, h0:h0 + 8].transpose([0, 2, 1])
            eng = nc.gpsimd if hg in PASSB_ON_GPSIMD else nc.vector
            eng.tensor_mul(ot[:, :, :], e_view, s["v"][:, h0:h0 + 8, :])
            nc.sync.dma_start(out=out[b, :, h0:h0 + 8, :], in_=ot[:])

        # ---- software pipeline ----
        # iteration i: QKproj(i) with att(i-1) interleaved, then sinv(i-1),
        # then Vproj(i-1) with pass2(i-1) chasing group-by-group.
        for i in range(B_LOC + 1):
            p = i if i < B_LOC else None            # QK projection batch
            c = i - 1 if i >= 1 else None           # att + V + pass2 batch

            if p is not None:
                emit_xload(p)
            for g in range(16):
                if c is not None and g < 8:
                    emit_att_group(c, g)
                if p is not None:
                    emit_qk_group(p, g)
            if c is not None:
                emit_sinv(c)
                for vg in range(16):
                    emit_v_group(c, vg)
                    emit_pass2(c, vg)

    nc.compile()
    return nc


def _get_built():
    if "nc" not in _BUILT:
        _BUILT["nc"] = _build_bass()
    return _BUILT["nc"]


def _prep_inputs(x, wq, bq, wk, bk, wv, bv, pos_code):
    x = np.asarray(x, np.float32)
    pos = np.asarray(pos_code, np.float32)[0]          # identical across channels
    waug = np.zeros([66, 192], np.float32)
    waug[0:64, 0:64] = np.asarray(wq, np.float32).T
    waug[0:64, 64:128] = np.asarray(wk, np.float32).T
    waug[0:64, 128:192] = np.asarray(wv, np.float32).T
    waug[64, 0:64] = np.asarray(bq, np.float32)
    waug[64, 64:128] = np.asarray(bk, np.float32)
    waug[64, 128:192] = np.asarray(bv, np.float32)
    waug[65, 64:128] = 1.0                             # P-row hits K channels only
    waug16 = waug.astype(np.float16)

    pflat16 = pos.reshape(-1).astype(np.float16)
    xf = x.reshape(x.shape[0], x.shape[1], S)
    in_maps = []
    for core in range(N_CORES):
        xs = xf[core * B_LOC:(core + 1) * B_LOC]
        xa = np.empty([B_LOC, 66, S], np.float16)
        xa[:, 0:64] = xs.astype(np.float16)
        xa[:, 64] = np.float16(1.0)
        xa[:, 65] = pflat16[None, :]
        in_maps.append({"xa": xa, "waug": waug16})
    return in_maps


LAST_RESULTS = None


def kernel(x, wq, bq, wk, bk, wv, bv, pos_code, _trace=False):
    global LAST_RESULTS
    in_maps = _prep_inputs(x, wq, bq, wk, bk, wv, bv, pos_code)
    nc = _get_built()
    res = run_bass_kernel_spmd(nc, in_maps, core_ids=list(range(N_CORES)),
                               trace=_trace)
    LAST_RESULTS = res
    outs = []
    for core in range(N_CORES):
        o = np.asarray(res.results[core]["out"])       # [4, w, h, c] bf16
        outs.append(np.transpose(o.astype(np.float32), (0, 3, 2, 1)))
    return np.concatenate(outs, axis=0)
